# revision 1
# baseline (speedup 1.0000x reference)
"""Laplacian normalization kernel for Trainium2 (8 NeuronCores, SPMD).

out = D^-1/2 A D^-1/2 where D = diag(row sums of A), A: [8192, 8192] fp32.

Sharding: rows split across 8 cores (1024 rows each). Per core:
  pass 1: stream stripes 0-3 first (quarter-width units), then load
    stripes 4-7 into RESIDENT SBUF tiles (16MB cache). Row sums reduce
    per unit; isq = 1/sqrt(deg) is finished per stripe.
  TWO AllGathers: AG1 ships stripes 0-3's isq chunks while stripes 4-7
    are still loading, AG2 ships the rest. AG1's output covers every
    output column j with (j mod 1024) < 512, so half of the scaling and
    stores run during the window where the kernel used to idle waiting
    on a single collective (which is bound by the slowest core).
  pass 2: out = (A * r[:,None]) * c[None,:], one fused DVE op per
    (unit, collective-half), strided over the covered column ranges.

Ring discipline: pass-2 reloads ride the sync HWDGE ring and stores ride
the scalar ring exclusively, so a store blocked on a post-collective
multiply can never sit ahead of an eligible reload in ring FIFO order.
Tiny latency-critical DMAs (isq writes, broadcasts) go via SWDGE.
"""

import sys

sys.path.insert(0, "/opt/trn_rl_repo")

import numpy as np

import concourse.bacc as bacc
import concourse.tile as tile
from concourse import mybir
from concourse.bass_utils import run_bass_kernel_spmd

N = 8192          # full matrix dim
CORES = 8
R = N // CORES    # rows per core: 1024
P = 128           # partitions
S = R // P        # row stripes per core: 8
HW = 4096         # resident half width
QW = 2048         # stream quarter width
NRES = 4          # stripes 4-7 resident in SBUF
HAG = R // 2      # isq elements per collective half: 512
F32 = mybir.dt.float32
MUL = mybir.AluOpType.mult
X = mybir.AxisListType.X

_CACHE = {}


def build_nc():
    if "nc" in _CACHE:
        return _CACHE["nc"]
    nc = bacc.Bacc(
        "TRN2", target_bir_lowering=False, debug=False, num_devices=CORES
    )
    a = nc.dram_tensor("a_block", [R, N], F32, kind="ExternalInput").ap()
    out = nc.dram_tensor("out_block", [R, N], F32, kind="ExternalOutput").ap()

    with tile.TileContext(nc) as tc:
        with (
            tc.tile_pool(name="dram", bufs=1, space="DRAM") as dram,
            tc.tile_pool(name="res", bufs=1) as res,
            tc.tile_pool(name="stream", bufs=4) as stream,
            tc.tile_pool(name="cpool", bufs=1) as cpool,
            tc.tile_pool(name="small", bufs=1) as small,
        ):
            # separate DRAM tensors per collective half so AG1's input
            # dependency can never couple to stripes 4-7's writes
            isq_loc = [
                dram.tile([HAG], F32, name=f"isq_loc{g}") for g in range(2)
            ]
            isq_ag = [
                dram.tile(
                    [CORES * HAG], F32, addr_space="Shared", name=f"isq_ag{g}"
                )
                for g in range(2)
            ]

            part = small.tile([P, 4 * S], F32)   # partial row sums
            isq_sb = small.tile([P, S], F32)     # per-stripe row scale

            def finish_stripe(s, nparts):
                """Combine partials -> isq -> isq_sb + DRAM chunk."""
                for i in range(1, nparts):
                    nc.vector.tensor_add(
                        part[:, 4 * s : 4 * s + 1],
                        part[:, 4 * s : 4 * s + 1],
                        part[:, 4 * s + i : 4 * s + i + 1],
                    )
                nc.vector.reciprocal(
                    part[:, 4 * s : 4 * s + 1], part[:, 4 * s : 4 * s + 1]
                )
                nc.scalar.sqrt(
                    isq_sb[:, s : s + 1], part[:, 4 * s : 4 * s + 1]
                )
                g, off = divmod(s * P, HAG)
                nc.gpsimd.dma_start(
                    isq_loc[g][off : off + P].unsqueeze(1),
                    isq_sb[:, s : s + 1],
                )

            # ---- pass 1 ----
            # streamed stripes 0-3 first: their isq feeds AG1, and their
            # reduces free the stream slots for pass-2 reloads early
            nunit = 0
            for s in range(S - NRES):
                for q in range(N // QW):
                    t = stream.tile([P, QW], F32, tag="stream")
                    ld = nc.sync if nunit % 2 == 0 else nc.scalar
                    ld.dma_start(
                        t[:], a[s * P : (s + 1) * P, q * QW : (q + 1) * QW]
                    )
                    nc.vector.reduce_sum(
                        out=part[:, 4 * s + q : 4 * s + q + 1], in_=t[:], axis=X
                    )
                    nunit += 1
                finish_stripe(s, N // QW)

            ag_args = dict(
                replica_groups=[list(range(CORES))],
            )
            nc.gpsimd.collective_compute(
                "AllGather",
                mybir.AluOpType.bypass,
                ins=[isq_loc[0][:].opt()],
                outs=[isq_ag[0][:].opt()],
                **ag_args,
            )

            # resident stripes 4-7, kept for pass 2
            res_tiles = {}
            for s in range(S - NRES, S):
                for h in range(N // HW):
                    t = res.tile([P, HW], F32, tag=f"res{s}_{h}", bufs=1)
                    ld = nc.sync if nunit % 2 == 0 else nc.scalar
                    ld.dma_start(
                        t[:], a[s * P : (s + 1) * P, h * HW : (h + 1) * HW]
                    )
                    nc.vector.reduce_sum(
                        out=part[:, 4 * s + h : 4 * s + h + 1], in_=t[:], axis=X
                    )
                    res_tiles[(s, h)] = t
                    nunit += 1
                finish_stripe(s, N // HW)

            nc.gpsimd.collective_compute(
                "AllGather",
                mybir.AluOpType.bypass,
                ins=[isq_loc[1][:].opt()],
                outs=[isq_ag[1][:].opt()],
                **ag_args,
            )

            # column-scale broadcast. AG half g covers, within each 1024
            # column block, columns [g*512, g*512+512). isq_ag[g] is
            # ordered (core, stripe-offset): element k*512 + u = isq of
            # global row k*1024 + g*512 + u = scale for that column.
            # cb[g][h] holds half g's scales for output columns
            # [h*4096, (h+1)*4096), packed compactly ([m*512+u] layout):
            # one tile per AG half, so the early multiplies can never
            # pick up a false dependency on the late collective.
            cb = [
                [
                    cpool.tile(
                        [P, HW // 2],
                        F32,
                        tag=f"cb{g}{h}",
                        bufs=1,
                        name=f"cb{g}{h}",
                    )
                    for h in range(N // HW)
                ]
                for g in range(2)
            ]
            for g in range(2):
                for h in range(N // HW):
                    src = (
                        isq_ag[g][h * (HW // 2) : (h + 1) * (HW // 2)]
                        .rearrange("(m c) -> m c", c=HAG)
                        .unsqueeze(0)
                        .to_broadcast([P, HW // 1024, HAG])
                    )
                    nc.gpsimd.dma_start(
                        cb[g][h][:].rearrange("p (m c) -> p m c", c=HAG), src
                    )

            # ---- pass 2: out = (A * r) * c ----
            def scale_store(s, col0, t, width, g):
                """Scale + store the AG-half-g columns of tile t."""
                h, hoff = divmod(col0, HW)
                m0 = hoff // 1024
                m = width // 1024
                c_ap = cb[g][h][
                    :, m0 * HAG : (m0 + m) * HAG
                ].rearrange("p (m c) -> p m c", c=HAG)
                nc.vector.scalar_tensor_tensor(
                    out=c3(t[:], 0, width, g),
                    in0=c3(t[:], 0, width, g),
                    scalar=isq_sb[:, s : s + 1],
                    in1=c_ap,
                    op0=MUL,
                    op1=MUL,
                )
                nc.scalar.dma_start(
                    c3(out[s * P : (s + 1) * P, :], col0, width, g),
                    c3(t[:], 0, width, g),
                )

            # resident stripes: AG1-covered columns first (those flow
            # while AG2 is still waiting on the slowest core)
            for s in range(S - NRES, S):
                for h in range(N // HW):
                    scale_store(s, h * HW, res_tiles[(s, h)], HW, 0)
            for s in range(S - NRES, S):
                for h in range(N // HW):
                    scale_store(s, h * HW, res_tiles[(s, h)], HW, 1)

            # streamed stripes reload on the sync ring, quarter width
            for s in range(S - NRES):
                for q in range(N // QW):
                    t = stream.tile([P, QW], F32, tag="stream")
                    nc.sync.dma_start(
                        t[:], a[s * P : (s + 1) * P, q * QW : (q + 1) * QW]
                    )
                    scale_store(s, q * QW, t, QW, 0)
                    scale_store(s, q * QW, t, QW, 1)

    nc.compile()
    _CACHE["nc"] = nc
    return nc


def c3(ap, col0, width, g):
    """The AG-half-g columns of ap's column range [col0, col0+width):
    within each 1024-column block, columns [g*512, g*512+512), as a
    strided [P, width//1024, 512] access pattern."""
    return ap[:, col0 : col0 + width].rearrange("p (m c) -> p m c", c=1024)[
        :, :, g * HAG : (g + 1) * HAG
    ]


def kernel(adjacency_matrix):
    A = np.ascontiguousarray(np.asarray(adjacency_matrix, dtype=np.float32))
    assert A.shape == (N, N)
    nc = build_nc()
    in_maps = [
        {"a_block": np.ascontiguousarray(A[k * R : (k + 1) * R])}
        for k in range(CORES)
    ]
    res = run_bass_kernel_spmd(nc, in_maps, list(range(CORES)))
    return np.concatenate(
        [res.results[k]["out_block"] for k in range(CORES)], axis=0
    )



# revision 3
# speedup vs baseline: 1.2835x; 1.2835x over previous
"""Laplacian normalization kernel for Trainium2 (8 NeuronCores, SPMD).

out = D^-1/2 A D^-1/2 where D = diag(row sums of A), A: [8192, 8192] fp32.

Sharding: core k owns global rows [k*512,(k+1)*512) u [4096+k*512, ...+512).
With that split, AllGather #1 (each core's first 512 isq values) yields the
contiguous global isq[0:4096] and AG2 yields isq[4096:8192] -- so column
scaling and stores run over contiguous column halves.

Single-read design: each stripe of A is loaded once (fp32, HWDGE). One fused
DVE tensor_scalar per half writes the bf16 resident copy (16MB in SBUF) AND
row-sum partials via accum_out. Pass 2 multiplies resident bf16 by the row
scale (per-partition scalar) and column scale (bf16 broadcast) into fp32
staging tiles and stores contiguously. Total HBM traffic = 64MB/core
(read 32 + write 32), the memory-bound minimum.

A warmup AllGather fires at t=0 to absorb collective-stream setup and launch
skew, so AG1/AG2 run at their ~17us steady-state latency.
"""

import sys

sys.path.insert(0, "/opt/trn_rl_repo")

import numpy as np

import concourse.bacc as bacc
import concourse.tile as tile
from concourse import mybir
from concourse.bass_utils import run_bass_kernel_spmd

N = 8192          # full matrix dim
CORES = 8
R = N // CORES    # rows per core: 1024
P = 128           # partitions
S = R // P        # row stripes per core: 8
HW = N // 2       # half width: 4096
HB = R // 2       # rows per collective half: 512
F32 = mybir.dt.float32
BF16 = mybir.dt.bfloat16
MUL = mybir.AluOpType.mult

_CACHE = {}


def build_nc():
    if "nc" in _CACHE:
        return _CACHE["nc"]
    nc = bacc.Bacc(
        "TRN2", target_bir_lowering=False, debug=False, num_devices=CORES
    )
    a = nc.dram_tensor("a_block", [R, N], F32, kind="ExternalInput").ap()
    out = nc.dram_tensor("out_block", [R, N], F32, kind="ExternalOutput").ap()

    with tile.TileContext(nc) as tc:
        with (
            tc.tile_pool(name="dram", bufs=1, space="DRAM") as dram,
            tc.tile_pool(name="res", bufs=1) as res,
            tc.tile_pool(name="io", bufs=3) as io,
            tc.tile_pool(name="cpool", bufs=1) as cpool,
            tc.tile_pool(name="small", bufs=1) as small,
        ):
            # per-collective-half DRAM tensors (collectives need internal DRAM)
            isq_loc = [
                dram.tile([HB], F32, name=f"isq_loc{g}") for g in range(2)
            ]
            isq_ag = [
                dram.tile(
                    [CORES * HB], F32, addr_space="Shared", name=f"isq_ag{g}"
                )
                for g in range(2)
            ]
            warm_loc = dram.tile([8], F32, name="warm_loc")
            warm_ag = dram.tile([CORES * 8], F32, addr_space="Shared",
                                name="warm_ag")

            part = small.tile([P, 2 * S], F32)   # row-sum partials (2/stripe)
            isq_sb = small.tile([P, S], F32)     # per-stripe row scale

            ag_args = dict(replica_groups=[list(range(CORES))])

            # warmup collective: absorbs CC-stream setup + launch skew so the
            # real AllGathers run at steady-state latency
            nc.gpsimd.collective_compute(
                "AllGather",
                mybir.AluOpType.bypass,
                ins=[warm_loc[:].opt()],
                outs=[warm_ag[:].opt()],
                **ag_args,
            )

            # ---- pass 1: load each stripe once, fused bf16-cast + row sum --
            res_t = []
            for s in range(S):
                t_res = res.tile([P, N], BF16, tag=f"res{s}", bufs=1)
                res_t.append(t_res)
                for h in range(2):
                    t = io.tile([P, HW], F32, tag="io")
                    ld = nc.sync if (2 * s + h) % 2 == 0 else nc.scalar
                    ld.dma_start(
                        t[:], a[s * P : (s + 1) * P, h * HW : (h + 1) * HW]
                    )
                    # resident bf16 copy + row-sum partial in one DVE op
                    nc.vector.tensor_scalar(
                        out=t_res[:, h * HW : (h + 1) * HW],
                        in0=t[:],
                        scalar1=1.0,
                        scalar2=None,
                        op0=MUL,
                        op1=mybir.AluOpType.add,
                        accum_out=part[:, 2 * s + h : 2 * s + h + 1],
                    )
                # finish stripe: deg -> isq = sqrt(1/deg)
                nc.vector.tensor_add(
                    part[:, 2 * s : 2 * s + 1],
                    part[:, 2 * s : 2 * s + 1],
                    part[:, 2 * s + 1 : 2 * s + 2],
                )
                nc.vector.reciprocal(
                    part[:, 2 * s : 2 * s + 1], part[:, 2 * s : 2 * s + 1]
                )
                nc.scalar.sqrt(
                    isq_sb[:, s : s + 1], part[:, 2 * s : 2 * s + 1]
                )
                g, off = divmod(s * P, HB)
                nc.gpsimd.dma_start(
                    isq_loc[g][off : off + P].unsqueeze(1),
                    isq_sb[:, s : s + 1],
                )
                if s == S // 2 - 1:
                    nc.gpsimd.collective_compute(
                        "AllGather",
                        mybir.AluOpType.bypass,
                        ins=[isq_loc[0][:].opt()],
                        outs=[isq_ag[0][:].opt()],
                        **ag_args,
                    )

            nc.gpsimd.collective_compute(
                "AllGather",
                mybir.AluOpType.bypass,
                ins=[isq_loc[1][:].opt()],
                outs=[isq_ag[1][:].opt()],
                **ag_args,
            )

            # column-scale broadcasts: isq_ag[g] is the contiguous global
            # isq[g*4096:(g+1)*4096]; replicate across partitions, cast bf16
            cb = [
                cpool.tile([P, HW], BF16, tag=f"cb{g}", bufs=1, name=f"cb{g}")
                for g in range(2)
            ]
            for g in range(2):
                nc.gpsimd.dma_start(
                    cb[g][:],
                    isq_ag[g][:].unsqueeze(0).to_broadcast([P, HW]),
                )

            # ---- pass 2: out = (bf16A * r) * c, contiguous column halves --
            for g in range(2):
                st = nc.sync if g == 0 else nc.scalar
                for s in range(S):
                    stg = io.tile([P, HW], F32, tag="io")
                    nc.vector.scalar_tensor_tensor(
                        out=stg[:],
                        in0=res_t[s][:, g * HW : (g + 1) * HW],
                        scalar=isq_sb[:, s : s + 1],
                        in1=cb[g][:],
                        op0=MUL,
                        op1=MUL,
                    )
                    st.dma_start(
                        out[s * P : (s + 1) * P, g * HW : (g + 1) * HW],
                        stg[:],
                    )

    nc.compile()
    _CACHE["nc"] = nc
    return nc


def _row_index(k):
    """Global row indices owned by core k, in local order."""
    return np.r_[k * HB : (k + 1) * HB, N // 2 + k * HB : N // 2 + (k + 1) * HB]


def make_in_maps(A):
    return [
        {"a_block": np.ascontiguousarray(A[_row_index(k)])}
        for k in range(CORES)
    ]


def unshard(results):
    out = np.empty((N, N), dtype=np.float32)
    for k in range(CORES):
        out[_row_index(k)] = results[k]["out_block"]
    return out


def kernel(adjacency_matrix):
    A = np.ascontiguousarray(np.asarray(adjacency_matrix, dtype=np.float32))
    assert A.shape == (N, N)
    nc = build_nc()
    res = run_bass_kernel_spmd(nc, make_in_maps(A), list(range(CORES)))
    return unshard(res.results)
